# revision 1
# baseline (speedup 1.0000x reference)
"""Single-head attention (B=4, Lq=Lkv=4096, D=128) on 8 TRN2 NeuronCores.

Sharding: data-parallel over (batch, query-half). Core c handles batch c//2,
query rows (c%2)*2048 ... +2048, with full K/V for that batch. No collectives.

Per-core kernel (all engines overlapped; ACT-exp is the steady-state rate):
  - Inputs stream in 512KB groups; PE-transposes x tiles (fp32 exact) with
    batched PSUM->SBUF copies split across DVE/ACT; projections in float32r
    (full PE rate, ~19-bit); V is projected to fp16 and DMA-transposed to
    natural [k, e] layout.
  - Main loop per (k-tile, q-half): S^T = K^T_tile.T @ Q^T (float32r) into
    PSUM; ACT computes exp (scale folded) -> E^T fp16 in SBUF; DVE
    accumulates E^T partial sums (fp16 2x mode, pair+chain); PE accumulates
    O^T += V_tile.T @ E^T in PSUM.
  - Epilogue per q-half: sumexp = all-ones matmul of the E^T sum (result
    replicated across partitions), reciprocal + multiply on DVE, chunked
    DMA out of O^T. The host transposes O^T -> O when stitching.

Numerics: scores in float32r (measured indistinguishable from fp32 here);
softmax without max-subtraction (|scores/sqrt(d)| < ~8, exp is safe in
fp32/fp16); P and V in fp16. End-to-end scale-relative absmax error vs the
fp32 reference: ~4e-4 (CoreSim and hardware).
"""

import os
import sys

# Recovers wedged NeuronCores (NRT_EXEC_UNIT_UNRECOVERABLE) at init; must be
# set before the first device use.
os.environ.setdefault("NEURON_RT_RESET_CORES", "1")

if "/opt/trn_rl_repo" not in sys.path:
    sys.path.insert(0, "/opt/trn_rl_repo")

from contextlib import ExitStack

import numpy as np

import concourse.bass as bass  # noqa: F401  (bass types used via bacc/tile)
import concourse.bacc as bacc
import concourse.tile as tile
from concourse import mybir
from concourse._compat import with_exitstack
from concourse.bass_utils import run_bass_kernel_spmd

D = 128
LQ = 2048  # per-core query slab
LKV = 4096
NQT = LQ // 128  # 16
NKT = LKV // 128  # 32
QH = 1024  # q chunk processed per pass (2 passes)
NCH = QH // 512  # 512-wide matmul chunks per pass
SCALE = float(1.0 / np.sqrt(128.0))

F32 = mybir.dt.float32
F32R = mybir.dt.float32r
BF16 = mybir.dt.bfloat16
FP16 = mybir.dt.float16


@with_exitstack
def attn_body(ctx: ExitStack, tc: tile.TileContext, io: dict):
    nc = tc.nc
    ctx.enter_context(
        nc.allow_low_precision(
            reason="f32r (19-bit) operands for full-rate PE matmul; fp32 PSUM accum"
        )
    )
    x1, x2, x3 = io["x1"], io["x2"], io["x3"]
    out = io["o"]

    # All constants arrive in one packed [128, 515] tensor (one DMA, issued
    # first): cols 0:128 Wq | 128:256 Wk | 256:384 Wv | 384:512 ident |
    # 512 bq | 513 bk | 514 bv.
    consts = ctx.enter_context(tc.tile_pool(name="consts", bufs=1))
    wpk = consts.tile([128, 515], F32)
    nc.sync.dma_start(out=wpk, in_=io["wpack"])
    w_nat = {"Wq": wpk[:, 0:128], "Wk": wpk[:, 128:256], "Wv": wpk[:, 256:384]}
    ident = wpk[:, 384:512]
    bias_t = {"Wq": wpk[:, 512:513], "Wk": wpk[:, 513:514], "Wv": wpk[:, 514:515]}
    ones_mat = consts.tile([128, 128], FP16)
    nc.vector.memset(ones_mat, 1.0)

    # ---- Phase 1: weight transposes, x transposes, projections ----
    # Persistent activations for the main loop. Quartered so Tile's
    # tile-granular dependency tracking lets the main loop start as soon as
    # the first quarter of K^T exists.
    acts = ctx.enter_context(tc.tile_pool(name="acts", bufs=1))
    qt_q = [acts.tile([128, QH], F32R, tag=f"qt{i}", name=f"qt{i}") for i in range(LQ // QH)]
    kt_q = [acts.tile([128, 1024], F32R, tag=f"kt{i}", name=f"kt{i}") for i in range(LKV // 1024)]
    vn_q = [
        acts.tile([128, 8, 128], FP16, tag=f"vn{i}", name=f"vn{i}") for i in range(NKT // 8)
    ]  # V natural [k%128, kt, e], quartered

    def kt_tile(kt):  # K^T 128-col block for k-tile kt
        return kt_q[kt // 8][:, (kt % 8) * 128 : (kt % 8 + 1) * 128]

    with (
        tc.tile_pool(name="wts", bufs=1) as wts,
        tc.tile_pool(name="xraw", bufs=2) as xraw,
        tc.tile_pool(name="xT", bufs=2) as xT,
        tc.tile_pool(name="ptr", bufs=2, space="PSUM") as ptr,
        tc.tile_pool(name="pmm", bufs=2, space="PSUM") as pmm,
        tc.tile_pool(name="vtmp", bufs=2) as vtmp,
    ):
        # Weights: PE-transpose the packed naturals to W^T [d, e].
        w_T = {}
        for name in ("Wq", "Wk", "Wv"):
            pt = ptr.tile([128, 128], F32, tag="ptrans")
            nc.tensor.transpose(pt, w_nat[name], ident)
            wt = wts.tile([128, 128], F32R, tag=f"wT_{name}")
            nc.vector.tensor_copy(out=wt, in_=pt)
            w_T[name] = wt

        # Each group = 8 x-tiles = 1024 columns: own DMA load, 8 PE
        # transposes into one [128, 1024] PSUM tile, one batched copy to
        # SBUF, projection matmuls + bias-add, all group-granular so the
        # main loop can start as soon as the first K^T quarter is ready.
        GRP = 8  # group size locked by kt quarter width

        def load_group(xin, name, g, on_act):
            raw = xraw.tile([128, GRP, 128], F32, tag=f"raw_{name}")
            nc.sync.dma_start(
                out=raw,
                in_=xin.rearrange("(t p) d -> p t d", p=128)[
                    :, g * GRP : (g + 1) * GRP, :
                ],
            )
            pt = ptr.tile([128, GRP * 128], F32, tag="ptrans")
            for j in range(GRP):
                nc.tensor.transpose(pt[:, j * 128 : (j + 1) * 128], raw[:, j, :], ident)
            xt_ = xT.tile([128, GRP * 128], F32R, tag=f"xT_{name}")
            if on_act:
                nc.scalar.copy(out=xt_, in_=pt)
            else:
                nc.vector.tensor_copy(out=xt_, in_=pt)
            return xt_

        def project_group(dst, wT, src, bias):
            ps = pmm.tile([128, 1024], F32, tag="proj")
            for h in range(2):
                nc.tensor.matmul(
                    ps[:, h * 512 : (h + 1) * 512],
                    wT,
                    src[:, h * 512 : (h + 1) * 512],
                    start=True,
                    stop=True,
                )
            nc.vector.tensor_scalar_add(out=dst, in0=ps, scalar1=bias)

        # Interleave groups so the main-loop critical path (Q^T half 0 and
        # K^T quarter 0, then V quarter 0) is produced first.
        work = [("x1", 0), ("x2", 0), ("x3", 0), ("x1", 1), ("x2", 1), ("x3", 1),
                ("x2", 2), ("x3", 2), ("x2", 3), ("x3", 3)]
        for name, g in work:
            if name == "x1":
                src = load_group(x1, "x1", g, on_act=False)
                project_group(qt_q[g], w_T["Wq"], src, bias_t["Wq"])
            elif name == "x2":
                src = load_group(x2, "x2", g, on_act=True)
                project_group(kt_q[g], w_T["Wk"], src, bias_t["Wk"])
            else:
                src = load_group(x3, "x3", g, on_act=True)
                vt = vtmp.tile([128, GRP * 128], FP16, tag="vT")
                project_group(vt, w_T["Wv"], src, bias_t["Wv"])
                nc.scalar.dma_start_transpose(out=vn_q[g], in_=vt)

    # ---- Phase 2: attention main loop ----
    otn_h = [acts.tile([128, QH], F32, tag=f"otn{i}", name=f"otn{i}") for i in range(LQ // QH)]
    with (
        tc.tile_pool(name="et", bufs=8) as etp,
        tc.tile_pool(name="sumt", bufs=6) as sumt,
        tc.tile_pool(name="stp", bufs=2, space="PSUM") as stp,
        tc.tile_pool(name="otp", bufs=2, space="PSUM") as otp,
        tc.tile_pool(name="nrm", bufs=2) as nrm,
    ):
        NQH = LQ // QH
        # kt-outer / qh-inner: K^T quarters are consumed at half the rate
        # (DMA keeps up during the ramp) and there is no mid-loop q-half
        # transition. Per-half E^T partial sums on DVE (fp16, 2x mode):
        # pairs -> linear chain of pairs, so the post-loop tail is short.
        ot_list = [otp.tile([128, QH], F32, tag="ot", name=f"ot{i}") for i in range(NQH)]
        pendings = [dict() for _ in range(NQH)]
        chains = [None] * NQH

        def sum_insert(qh, tile_):
            pending = pendings[qh]
            if 0 not in pending:
                pending[0] = tile_
                return
            prev = pending.pop(0)
            pair = sumt.tile([128, QH], FP16, tag="sum0", name="s0")
            nc.vector.tensor_add(out=pair, in0=prev, in1=tile_)
            if chains[qh] is None:
                chains[qh] = pair
            else:
                acc = sumt.tile([128, QH], FP16, tag="sumc", name="sc")
                nc.vector.tensor_add(out=acc, in0=chains[qh], in1=pair)
                chains[qh] = acc

        def iteration(kt, qh):
            st = stp.tile([128, QH], F32, tag="st", name="st")
            for c in range(NCH):
                sl = slice(c * 512, (c + 1) * 512)
                nc.tensor.matmul(
                    st[:, sl],
                    kt_tile(kt),
                    qt_q[qh][:, c * 512 : (c + 1) * 512],
                    start=True,
                    stop=True,
                )
            et = etp.tile([128, QH], FP16, tag="et", name="et")
            nc.scalar.activation(
                out=et, in_=st, func=mybir.ActivationFunctionType.Exp, scale=SCALE
            )
            sum_insert(qh, et)
            for c in range(NCH):
                sl = slice(c * 512, (c + 1) * 512)
                nc.tensor.matmul(
                    ot_list[qh][:, sl],
                    vn_q[kt // 8][:, kt % 8, :],
                    et[:, sl],
                    start=kt == 0,
                    stop=kt == NKT - 1,
                )

        def epilogue(qh):
            q0 = qh * QH
            esum = chains[qh]
            # Partition-reduce esum with an all-ones [128,128] stationary so
            # the result lands replicated across partitions (no broadcast);
            # recip/mul/DMA chunked so the chain pipelines. se borrows an st
            # slot; the normalize multiply reads O^T PSUM directly.
            se_ps = stp.tile([128, QH], F32, tag="st", name="se")
            for c in range(NCH):
                sl = slice(c * 512, (c + 1) * 512)
                nc.tensor.matmul(
                    se_ps[:, sl], ones_mat, esum[:, sl], start=True, stop=True
                )
                rec = nrm.tile([128, 512], F32, tag="rec", name="rec")
                nc.vector.reciprocal(out=rec, in_=se_ps[:, sl])
                nc.vector.tensor_mul(
                    out=otn_h[qh][:, sl], in0=ot_list[qh][:, sl], in1=rec
                )
                nc.sync.dma_start(
                    out=out[:, q0 + c * 512 : q0 + (c + 1) * 512],
                    in_=otn_h[qh][:, sl],
                )

        for i in range(NKT):
            iteration(i, 0)
            iteration(i, 1)
        epilogue(0)
        epilogue(1)


def build_nc() -> "bacc.Bacc":
    nc = bacc.Bacc("TRN2", target_bir_lowering=False, debug=False, num_devices=8)
    io = {}
    io["x1"] = nc.dram_tensor("x1", [LQ, D], F32, kind="ExternalInput").ap()
    io["x2"] = nc.dram_tensor("x2", [LKV, D], F32, kind="ExternalInput").ap()
    io["x3"] = nc.dram_tensor("x3", [LKV, D], F32, kind="ExternalInput").ap()
    io["wpack"] = nc.dram_tensor("wpack", [128, 515], F32, kind="ExternalInput").ap()
    io["o"] = nc.dram_tensor("o", [128, LQ], F32, kind="ExternalOutput").ap()
    with tile.TileContext(nc) as tc:
        attn_body(tc, io)
    nc.compile()
    return nc


def make_in_maps(inputs: dict) -> list[dict]:
    wpack = np.concatenate(
        [
            np.asarray(inputs["Wq"], np.float32),
            np.asarray(inputs["Wk"], np.float32),
            np.asarray(inputs["Wv"], np.float32),
            np.eye(128, dtype=np.float32),
            np.asarray(inputs["bq"], np.float32)[:, None],
            np.asarray(inputs["bk"], np.float32)[:, None],
            np.asarray(inputs["bv"], np.float32)[:, None],
        ],
        axis=1,
    )
    shared = {"wpack": np.ascontiguousarray(wpack)}
    x1 = np.asarray(inputs["x1"], np.float32)
    x2 = np.asarray(inputs["x2"], np.float32)
    x3 = np.asarray(inputs["x3"], np.float32)
    in_maps = []
    for c in range(8):
        b, qh = c // 2, c % 2
        in_maps.append(
            {
                "x1": np.ascontiguousarray(x1[b, qh * LQ : (qh + 1) * LQ, :]),
                "x2": np.ascontiguousarray(x2[b]),
                "x3": np.ascontiguousarray(x3[b]),
                **shared,
            }
        )
    return in_maps


_NC_CACHE = None


def get_nc():
    global _NC_CACHE
    if _NC_CACHE is None:
        _NC_CACHE = build_nc()
    return _NC_CACHE


def kernel(**inputs) -> np.ndarray:
    nc = get_nc()
    in_maps = make_in_maps(inputs)
    res = run_bass_kernel_spmd(nc, in_maps, core_ids=list(range(8)))
    out = np.empty((4, 4096, 128), np.float32)
    for c in range(8):
        b, qh = c // 2, c % 2
        out[b, qh * LQ : (qh + 1) * LQ, :] = res.results[c]["o"].T
    return out


if __name__ == "__main__":
    nc = build_nc()
    print("built OK")

